# revision 11
# baseline (speedup 1.0000x reference)
"""Trainium2 Bass kernel for nn_BoxLoss (YOLO-style box regression loss).

Contract: kernel(**inputs) takes FULL unsharded inputs (numpy), returns the
FULL scalar loss. Internally: pure data parallel over batch across 8
NeuronCores (4 images per core); each core computes its 12 (scale, image)
row losses entirely on-device and writes a [2,1] partial; the host adds
the 16 partials while unsharding.

Layout: 128 partitions, p = bh*64 + j (image-half, target slot); slots
j in [50,64) are benign padding so the per-bh-half matmuls can write at
base partitions 0 and 64 (the only legal matmul output bases besides 32).
Free dim sbl = s*2 + bl (scale, image-parity), channels innermost.

v2 vs the original baseline:
- ONE merged indirect gather (768 descriptors) instead of 6 serialized
  SWDGE instructions - the ~1us fixed SWDGE overhead is paid once.
- last-wins dedup broadcast runs entirely on the PE: transpose matmul,
  block-diagonal mask, then per-bh-half indicator matmuls back to
  [p, (sbl, k)] - no DRAM roundtrip, no int16 keys. Keys are the f32
  gather indices (< 2^23, exact through the 0/1 indicator matmuls).
- floor via AluOp mod; anchor argmax and the *85 index math fused; the
  three per-scale post-gather chains merged into one wide chain.
- elementwise work split across DVE and Pool so the serial chain that
  feeds the gather is shorter.
"""

import numpy as np

import concourse.bass as bass
import concourse.bacc as bacc
import concourse.mybir as mybir
import concourse.tile as tile

NCORES = 8
GRIDS = (52, 26, 13)
A = 3           # anchors per scale
T = 50          # targets per image
PB = 4          # images per core
B_TOTAL = 32
HALF = 64       # partition stride of the bh halves
P = 128         # partitions: (bh, j) with 14 pad slots per half
SBL = 6         # free rows: (s, bl)
BIG = float(2 ** 13)   # sentinel key for unmatched targets (> any cell id)

F32 = mybir.dt.float32
I32 = mybir.dt.int32

_SCALE_ELEMS = [PB * A * g * g * 85 for g in GRIDS]
_SCALE_BASE = [0, _SCALE_ELEMS[0], _SCALE_ELEMS[0] + _SCALE_ELEMS[1]]
OUTCAT_ELEMS = sum(_SCALE_ELEMS)

# hostpack column layout ([128, _HP_TOT])
_H_TGT = 0        # [0,8)    raw targets (bl, c)
_H_AWH = 8        # [8,44)   anchor w/h (q, sbl, a)
_H_G24 = 44       # [44,68)  g per (sbl, c)
_H_BG = 68        # [68,74)  scale base + b*3*g^2*85  (b = 2bh+bl)
_H_W85 = 74       # [74,80)  85*g
_H_HW85 = 80      # [80,86)  85*g^2
_HP_TOT = 86

# cc inline const [128, _C_TOT]: lat300 | EYE128 | onesU
_C_LAT = 0        # [0,300)   lat[p, sbl*50+k] = (k > j(p))
_C_EYE = 300      # [300,428) identity-128
_C_ONESU = 428    # [428,430) per-half real-row indicators
_C_TOT = 430

# em inline const [6, _E_TOT]: blkmask 600 (bh, sblk, k) | ones 50
_E_BLK = 0
_E_ONES = 600
_E_TOT = 656


def _host_consts():
    sbl = np.arange(SBL)
    s = sbl // 2
    g = np.array(GRIDS, dtype=np.float64)[s]              # [6]

    g24 = np.broadcast_to(g[:, None], (SBL, 4)).reshape(-1)       # [24]
    w85 = 85.0 * g
    hw85 = 85.0 * g * g
    p = np.arange(P)
    bh = p // HALF
    j = p % HALF
    base = np.array(_SCALE_BASE, dtype=np.float64)[s][None, :]
    b = (2 * bh[:, None] + (sbl % 2)[None, :])
    bg = base + b * (A * 85) * (g ** 2)[None, :]          # [128, 6]
    bg[j >= T, :] = 0.0                                   # pad rows

    row = np.concatenate([
        np.zeros(8), np.zeros(36), g24, np.zeros(6), w85, hw85])
    hp_const = np.broadcast_to(row, (P, _HP_TOT)).copy()
    hp_const[:, _H_BG:_H_BG + 6] = bg
    # benign pad targets: x=0 (invalid), wh=1 (finite rsqrt chain)
    pad = np.tile(np.array([0.0, 0.0, 1.0, 1.0], np.float64), 2)
    hp_const[j >= T, _H_TGT:_H_TGT + 8] = pad
    return hp_const.astype(np.float32)


def _inline_consts():
    p = np.arange(P)
    j = (p % HALF)[:, None]
    k = np.tile(np.arange(T), SBL)[None, :]
    cc = np.zeros((P, _C_TOT), np.float32)
    cc[:, _C_LAT:_C_LAT + SBL * T] = (k > j)
    cc[:, _C_EYE:_C_EYE + P] = np.eye(P, dtype=np.float32)
    jf = p % HALF
    cc[(p < HALF) & (jf < T), _C_ONESU] = 1.0
    cc[(p >= HALF) & (jf < T), _C_ONESU + 1] = 1.0

    # em[s, (bh, sblk, k)] = (s == sblk); em[:, 600:650] = 1
    em = np.zeros((SBL, _E_TOT), np.float32)
    blk = np.zeros((SBL, 2, SBL, T), np.float32)
    for s_ in range(SBL):
        blk[s_, :, s_, :] = 1.0
    em[:, _E_BLK:_E_BLK + 600] = blk.reshape(SBL, 600)
    em[:, _E_ONES:_E_TOT] = 1.0
    return np.ascontiguousarray(cc), np.ascontiguousarray(em)


def build_nc(use_collective: bool = False):
    nc = bacc.Bacc("TRN2", target_bir_lowering=False, debug=False,
                   num_devices=NCORES)

    hp_d = nc.dram_tensor("hostpack", [P, _HP_TOT], F32, kind="ExternalInput")
    outcat_d = nc.dram_tensor("outcat", [OUTCAT_ELEMS], F32, kind="ExternalInput")
    loss_d = nc.dram_tensor("loss", [2, 1], F32, kind="ExternalOutput")
    cc_np, em_np = _inline_consts()
    cc_d = nc.inline_tensor(cc_np, name="cc")
    em_d = nc.inline_tensor(em_np, name="em")

    AL = mybir.AluOpType
    AX = mybir.AxisListType.X

    with tile.TileContext(nc) as tc:
        with (
            tc.tile_pool(name="sbuf", bufs=1) as sp,
            tc.tile_pool(name="psum", bufs=1, space="PSUM") as pp,
        ):
            V = nc.vector
            G = nc.gpsimd

            def tt(eng, out, in0, in1, op):
                return eng.tensor_tensor(out=out, in0=in0, in1=in1, op=op)

            def ts(eng, out, in0, s1, op, s2=None, op2=None):
                if op2 is None:
                    return eng.tensor_scalar(out=out, in0=in0, scalar1=s1,
                                             scalar2=None, op0=op)
                return eng.tensor_scalar(out=out, in0=in0, scalar1=s1,
                                         scalar2=s2, op0=op, op1=op2)

            def stt(eng, out, in0, scalar, in1, op0, op1):
                return eng.scalar_tensor_tensor(
                    out=out, in0=in0, scalar=scalar, in1=in1, op0=op0, op1=op1)

            _tn = [0]

            def new(shape, dt=F32):
                _tn[0] += 1
                return sp.tile(shape, dt, name=f"t{_tn[0]}")

            # ---------- input loads ----------
            hp = new([P, _HP_TOT])
            nc.sync.dma_start(out=hp[:], in_=hp_d[:, :])
            cc = new([P, _C_TOT])
            nc.scalar.dma_start(out=cc[:], in_=cc_d[:, :])
            em = new([SBL, _E_TOT])
            nc.scalar.dma_start(out=em[:], in_=em_d[:, :])

            def C(c0, w):
                return hp[:, c0:c0 + w]

            tgt = C(_H_TGT, 8)
            awh2 = C(_H_AWH, 36)
            EYE = cc[:, _C_EYE:_C_EYE + P]
            onesU = cc[:, _C_ONESU:_C_ONESU + 2]
            lat = cc[:, _C_LAT:_C_LAT + SBL * T]

            # ---------- Pool: validity + anchor prep ----------
            # padding rows are all-zero; real rows have x in (0.02, 0.98),
            # so x > 0 is an exact validity test for this input family.
            v2 = new([P, 2])
            ts(G, v2[:], tgt.rearrange("p (bl c) -> p bl c", c=4)[:, :, 0:1],
               0.0, AL.is_gt)
            awhh = new([P, 36]); ts(G, awhh[:], awh2, 0.5, AL.mult)
            nawhh = new([P, 36]); ts(G, nawhh[:], awh2, -0.5, AL.mult)
            areaa = new([P, 18])
            tt(G, areaa[:], awh2[:, 0:18], awh2[:, 18:36], AL.mult)

            # ---------- DVE: t = raw * g ----------
            t4 = new([P, 24])
            tt(V, t4[:], tgt[:, None, :].to_broadcast([P, 3, 8]),
               C(_H_G24, 24), AL.mult)
            t4v = t4[:].rearrange("p (sbl c) -> p sbl c", c=4)
            txy = t4v[:, :, 0:2]
            twh = t4v[:, :, 2:4]

            # ---------- floor via round-to-nearest + correction ----------
            r2 = new([P, 12])
            ts(V, r2[:], txy, float(2 ** 23), AL.add, -float(2 ** 23), AL.add)
            gtm = new([P, 12])
            tt(V, gtm[:], r2[:], txy, AL.is_gt)

            # Pool: fxy = floor(txy) and the index partial iw = 85cx+bg+85g*cy
            fxy = new([P, 12])
            tt(G, fxy[:], r2[:], gtm[:], AL.subtract)
            zt05 = new([P, 12])
            stt(V, zt05[:], txy, -0.5, fxy[:], AL.add, AL.subtract)
            fv = fxy[:].rearrange("p (sbl q) -> p sbl q", q=2)
            cx = fv[:, :, 0:1]
            cy = fv[:, :, 1:2]
            iu1 = new([P, 6])
            ts(G, iu1[:], cx, 85.0, AL.mult)
            iu = new([P, 6])
            tt(G, iu[:], iu1[:], C(_H_BG, 6), AL.add)
            iv_ = new([P, 6])
            tt(G, iv_[:], cy, C(_H_W85, 6), AL.mult)
            iw = new([P, 6])
            tt(G, iw[:], iu[:], iv_[:], AL.add)

            # Pool: area of target boxes + union partial
            areat = new([P, 6])
            tt(G, areat[:], t4v[:, :, 2:3], t4v[:, :, 3:4], AL.mult)
            un1 = new([P, 18])
            tt(G, un1[:], areat[:, :, None].to_broadcast([P, SBL, 3]),
               areaa[:], AL.add)

            # ---------- DVE: IoU in (q, sbl, a) layout ----------
            lo = new([P, 12])
            stt(V, lo[:], twh, -0.5, zt05[:], AL.mult, AL.add)
            hi = new([P, 12])
            stt(V, hi[:], twh, 0.5, zt05[:], AL.mult, AL.add)

            def bcQ(t12):
                return (t12[:].rearrange("p (sbl q) -> p q sbl", q=2)
                        [:, :, :, None].to_broadcast([P, 2, SBL, 3]))

            P0 = new([P, 36]); tt(V, P0[:], bcQ(lo), nawhh[:], AL.max)
            P1 = new([P, 36]); tt(V, P1[:], bcQ(hi), awhh[:], AL.min)
            D = new([P, 36]); tt(V, D[:], P1[:], P0[:], AL.subtract)
            M0 = new([P, 36]); ts(V, M0[:], D[:], 0.0, AL.max)
            inter = new([P, 18])
            tt(V, inter[:], M0[:, 0:18], M0[:, 18:36], AL.mult)
            union = new([P, 18])
            tt(V, union[:], un1[:], inter[:], AL.subtract)
            runi = new([P, 18]); V.reciprocal(out=runi[:], in_=union[:])
            iou = new([P, 18]); tt(V, iou[:], inter[:], runi[:], AL.mult)

            # ---------- DVE: overlap / argmax / gather index ----------
            iv = iou[:].rearrange("p (sbl a) -> p sbl a", a=3)
            overlap = new([P, 6])
            V.reduce_max(out=overlap[:], in_=iv, axis=AX)
            eqB = new([P, 12])
            tt(V, eqB[:], iv[:, :, 0:2],
               overlap[:, :, None].to_broadcast([P, SBL, 2]), AL.is_equal)
            ev = eqB[:].rearrange("p (sbl e) -> p sbl e", e=2)
            t2 = new([P, 6])
            ts(V, t2[:], ev[:, :, 1:2], 0.0, AL.is_equal, 1.0, AL.add)
            anc = new([P, 6])
            stt(V, anc[:], ev[:, :, 0:1], 0.0, t2[:], AL.is_equal, AL.mult)
            ca = new([P, 6])
            tt(V, ca[:], anc[:], C(_H_HW85, 6), AL.mult)
            idxf = new([P, 6])
            tt(V, idxf[:], ca[:], iw[:], AL.add)

            # rsqrt of t_wh (DVE + ACT; off the gather path)
            rwh2 = new([P, 12])
            V.reciprocal(out=rwh2[:], in_=twh)
            rstw = new([P, 12]); nc.scalar.sqrt(out=rstw[:], in_=rwh2[:])

            # ---------- Pool: match mask, int index, ONE merged gather ----
            om = new([P, 6]); ts(G, om[:], overlap[:], 0.5, AL.is_gt)
            m = new([P, 6])
            tt(G, m[:], om[:].rearrange("p (s bl) -> p s bl", bl=2),
               v2[:, None, :].to_broadcast([P, 3, 2]), AL.mult)
            idxi = new([P, 6], I32)
            G.tensor_copy(out=idxi[:], in_=idxf[:])
            g24t = new([P, 24])
            G.indirect_dma_start(
                out=g24t[:], out_offset=None,
                in_=outcat_d[:].unsqueeze(1),
                in_offset=bass.IndirectOffsetOnAxis(ap=idxi[:, 0:6], axis=0),
            )

            # Pool: dedup keys (key = cell-scale id if matched else SENT).
            # (idx - bg)/85 stays ~= cell id (< 2^13): small enough to pass
            # bit-exactly through the PE indicator matmuls below.
            cs = new([P, 6])
            tt(G, cs[:], idxf[:], C(_H_BG, 6), AL.subtract)
            ck = new([P, 6])
            ts(G, ck[:], cs[:], 1.0 / 85.0, AL.mult)
            kk1 = new([P, 6])
            ts(G, kk1[:], ck[:], -BIG, AL.add)
            kk = new([P, 6])
            tt(G, kk[:], kk1[:], m[:], AL.mult)
            key = new([P, 6]); ts(G, key[:], kk[:], BIG, AL.add)

            # ---------- PE dedup broadcast ----------
            keyT_p = pp.tile([SBL, P], F32, name="keyT_p")
            nc.tensor.matmul(out=keyT_p[:], lhsT=key[:], rhs=EYE,
                             start=True, stop=True)
            # blk[s, (bh, sblk, k)] = keyT[s, bh*64+k] * (s == sblk)
            blk = new([SBL, 600])
            tt(V, blk[:],
               keyT_p[:].rearrange("s (bh k) -> s bh k", bh=2)
               [:, :, 0:T][:, :, None, :].to_broadcast([SBL, 2, SBL, T]),
               em[:, _E_BLK:_E_BLK + 600], AL.mult)
            # keyB[p, (sbl, k)] = key[bh(p)*64+k, sbl] via per-half indicator
            kb_p = pp.tile([P, SBL * T], F32, name="kb_p")
            ones6 = em[:, _E_ONES:_E_ONES + T]
            nc.tensor.matmul(out=kb_p[0:T, :], lhsT=ones6,
                             rhs=blk[:, 0:300], start=True, stop=True)
            nc.tensor.matmul(out=kb_p[HALF:HALF + T, :], lhsT=ones6,
                             rhs=blk[:, 300:600], start=True, stop=True)

            # ---------- DVE: dedup compare (fills the gather window) ------
            E = new([P, SBL * T])
            tt(V, E[:], key[:, :, None].to_broadcast([P, SBL, T]),
               kb_p[:].rearrange("p (sbl k) -> p sbl k", k=T), AL.is_equal)
            EL = new([P, SBL * T])
            tt(V, EL[:], E[:], lat, AL.mult)
            ov = new([P, 6])
            V.reduce_max(out=ov[:],
                         in_=EL[:].rearrange("p (sbl k) -> p sbl k", k=T),
                         axis=AX)
            winner2 = new([P, 12])  # cols 0:6 winner, cols 6:12 winner*TS
            nov = new([P, 6]); ts(G, nov[:], ov[:], 0.0, AL.is_equal)
            tt(G, winner2[:, 0:6], m[:], nov[:], AL.mult)

            # ---------- post-gather: one wide residual chain ----------
            gv = g24t[:].rearrange("p (sbl c) -> p sbl c", c=4)
            rcpw = new([P, 12])
            V.reciprocal(out=rcpw[:], in_=gv[:, :, 2:4])
            rspw = new([P, 12]); nc.scalar.sqrt(out=rspw[:], in_=rcpw[:])
            sel = new([P, 24])
            selv = sel[:].rearrange("p (sbl c) -> p sbl c", c=4)
            tt(V, selv[:, :, 0:2], gv[:, :, 0:2], txy, AL.subtract)
            tt(V, selv[:, :, 2:4], rspw[:].rearrange("p (sbl q) -> p sbl q", q=2),
               rstw[:].rearrange("p (sbl q) -> p sbl q", q=2), AL.subtract)
            sq = new([P, 24]); tt(V, sq[:], sel[:], sel[:], AL.mult)
            TS2 = new([P, 6])
            V.reduce_sum(out=TS2[:],
                         in_=sq[:].rearrange("p (sbl c) -> p sbl c", c=4),
                         axis=AX)
            tt(V, winner2[:, 6:12], TS2[:], winner2[:, 0:6], AL.mult)

            # ---------- partition reduce + per-row normalize ----------
            M1_p = pp.tile([2, 12], F32, name="M1_p")
            nc.tensor.matmul(out=M1_p[:], lhsT=onesU, rhs=winner2[:],
                             start=True, stop=True)
            mx2 = new([2, 6])
            ts(V, mx2[:], M1_p[:, 0:6], 1.0, AL.max, 2.0, AL.mult)
            rden2 = new([2, 6]); V.reciprocal(out=rden2[:], in_=mx2[:])
            rl2 = new([2, 6])
            stt(V, rl2[:], M1_p[:, 6:12], 1.0 / B_TOTAL, rden2[:],
                AL.mult, AL.mult)
            pt2 = new([2, 1])
            V.reduce_sum(out=pt2[:], in_=rl2[:], axis=AX)
            nc.sync.dma_start(out=loss_d[:, :], in_=pt2[:])

    nc.compile()
    return nc


_HOST_CONSTS = _host_consts()


def make_in_maps(output0, anchors0, output1, anchors1, output2, anchors2,
                 targets):
    outs = [np.asarray(output0), np.asarray(output1), np.asarray(output2)]
    ancs = [np.asarray(anchors0), np.asarray(anchors1), np.asarray(anchors2)]
    tg = np.asarray(targets)

    # anchor block (q, sbl, a): col = q*18 + (s*2+bl)*3 + a
    awh_row = np.zeros(36, np.float32)
    for q_, col in ((0, 0), (1, 1)):
        for s_ in range(3):
            for bl in range(2):
                for a_ in range(3):
                    awh_row[q_ * 18 + (s_ * 2 + bl) * 3 + a_] = ancs[s_][a_, col]

    in_maps = []
    for c in range(NCORES):
        sl = slice(c * PB, (c + 1) * PB)
        raw = tg[sl, :, 1:5].astype(np.float32)          # [4, 50, 4]
        tg8 = (raw.reshape(2, 2, T, 4)                    # (bh, bl, j, c)
               .transpose(0, 2, 1, 3).reshape(2, T, 8))   # (bh, j) x (bl,c)
        hostpack = _HOST_CONSTS.copy()
        hostpack[0:T, _H_TGT:_H_TGT + 8] = tg8[0]
        hostpack[HALF:HALF + T, _H_TGT:_H_TGT + 8] = tg8[1]
        hostpack[:, _H_AWH:_H_AWH + 36] = awh_row[None, :]
        outcat = np.concatenate([o[sl].ravel() for o in outs]).astype(np.float32)
        in_maps.append({"hostpack": np.ascontiguousarray(hostpack),
                        "outcat": outcat})
    return in_maps


_NC_CACHE = {}


def kernel(output0, anchors0, output1, anchors1, output2, anchors2, targets):
    import time
    from concourse.bass_utils import run_bass_kernel_spmd

    if "nc" not in _NC_CACHE:
        _NC_CACHE["nc"] = build_nc(use_collective=False)
    nc = _NC_CACHE["nc"]
    in_maps = make_in_maps(output0, anchors0, output1, anchors1, output2,
                           anchors2, targets)
    res = None
    for attempt in range(3):
        try:
            res = run_bass_kernel_spmd(nc, in_maps, list(range(NCORES)))
            break
        except Exception:
            # transient NRT device errors have been observed; back off + retry
            if attempt == 2:
                raise
            time.sleep(20.0 * (attempt + 1))
    total = np.float32(0.0)
    for c in range(NCORES):
        total += np.float32(np.asarray(res.results[c]["loss"]).sum())
    return np.float32(total)
